# revision 9
# baseline (speedup 1.0000x reference)
"""CapsuleLayer (dynamic routing) Trainium2 kernel, v2.

Math (per example a):
  H[a,b,c,j] = sum_i x[a,c,i] * W[b,c,j,i]          (inputs_hat)
  3 routing iterations of:
    coef = softmax_b(L); s = sum_c coef*H; out = squash(s); L += sum_d out*H

Distribution: data-parallel over batch, 512 = 8 cores x 64 examples.

Per-core layout: SBUF partition p = b0*64 + a (b0 = capsule half, a = local
example). H is stored bf16 [128, (c144, d16, b16)] with b innermost: every
big routing op (products with (b,c)- or (d,b)-shaped multipliers, the c- and
d-reduction trees) then has a stride-1 innermost run and hits DVE 2x mode
without any operand expansion.

s0 (uniform-coefficient round) never touches H: it is a K=32 PE contraction
sum_{c,i} x*W accumulated over 36 chunked matmuls into one PSUM bank, using
a second compact copy of W ([32, .] partitions) and a b0-duplicated x
stationary (m=128) so the PSUM drain is same-partition.

W streams in 4 chunks; H matmuls + drains + the iter-0 b-update pipeline
behind the stream, spread over SP/Act (DMA), PE (matmuls), Pool/DVE/Act
(drains) and DVE/Pool (products, trees).
"""

import sys

for _p in ("/opt/trn_rl_repo",):
    if _p not in sys.path:
        sys.path.insert(0, _p)

from contextlib import ExitStack

import numpy as np

import concourse.bass as bass
import concourse.mybir as mybir
from concourse import tile
from concourse.bass_utils import run_bass_kernel_spmd

F32 = mybir.dt.float32
BF16 = mybir.dt.bfloat16
AF = mybir.ActivationFunctionType
ALU = mybir.AluOpType
AX = mybir.AxisListType

B = 512
NCORES = 8
BS = B // NCORES  # 64 examples per core
NCAP = 32
B16 = 16  # capsules per half
CIN = 144
CQ = 36  # c // 4
D = 16
I8 = 8
EPS = 1e-7
ROUTINGS = 3

HFREE = CIN * D * B16  # 36864 elements per partition
CSPLIT = 88  # c in [0, CSPLIT) on DVE, [CSPLIT, 144) on Pool


def _halving_tree(eng, view, lo, width, unit):
    """Fold view[:, lo:lo+width, ...] into view[:, lo:lo+1, ...] by repeated
    halving along the c axis (dim 1). `unit` unused; kept for clarity."""
    w = width
    while w > 1:
        half = w // 2
        eng.tensor_tensor(
            view[:, lo : lo + half],
            view[:, lo : lo + half],
            view[:, lo + half : lo + 2 * half],
            op=ALU.add,
        )
        if w % 2 == 1:
            eng.tensor_tensor(
                view[:, lo : lo + 1],
                view[:, lo : lo + 1],
                view[:, lo + w - 1 : lo + w],
                op=ALU.add,
            )
        w = half


def _build_program() -> bass.Bass:
    nc = bass.Bass()
    ilhs_d = nc.declare_dram_parameter("ilhs", [32, CQ * BS], BF16, isOutput=False)
    ilhs2_d = nc.declare_dram_parameter("ilhs2", [32, CQ * 128], BF16, isOutput=False)
    wrhs_d = nc.declare_dram_parameter("wrhs", [32, CQ * 512], BF16, isOutput=False)
    out_d = nc.declare_dram_parameter("out", [BS, NCAP, D], F32, isOutput=True)

    with ExitStack() as ctx:
        tc = ctx.enter_context(tile.TileContext(nc))
        cpool = ctx.enter_context(tc.tile_pool(name="const", bufs=1))

        H_t = cpool.tile([128, HFREE], BF16)
        prod = cpool.tile([128, HFREE], BF16)
        ilhs_t = cpool.tile([128, CQ * BS], BF16)
        ilhs2_t = cpool.tile([32, CQ * 128], BF16)

        # persistent small tensors
        s_t = cpool.tile([128, 256], F32)  # (d, b)
        L_t = cpool.tile([128, CIN * B16], F32)  # logits (c, b)
        E_t = cpool.tile([128, CIN * B16], BF16)  # exp(L) (c, b)
        Lh_t = E_t  # Lhat scratch reuses E (E is dead after the C-multiply)
        C_t = cpool.tile([128, CIN * B16], BF16)  # coefficients (c, b)
        Dt8 = cpool.tile([128, CIN * 8], BF16)  # denom tree scratch (c, 8)
        Dh = cpool.tile([128, CIN], BF16)
        Dtmp = cpool.tile([128, CIN], BF16)
        Rh = cpool.tile([128, CIN], BF16)
        Rh2 = cpool.tile([128, CIN * 2], BF16)  # recip denom (c, 2)
        sq = cpool.tile([128, 256], F32)
        n2 = cpool.tile([128, B16], F32)
        t1 = cpool.tile([128, B16], F32)
        r1 = cpool.tile([128, B16], F32)
        rs = cpool.tile([128, B16], F32)
        fac = cpool.tile([128, B16], F32)
        outB = cpool.tile([128, 256], BF16)  # squashed out (d, b)
        outF = cpool.tile([128, 256], F32)  # final out (b, d)
        epsb = cpool.tile([128, 1], F32)
        tpre = cpool.tile([128, 1], F32)
        nc.vector.memset(epsb[:], EPS)
        # preload the natural_log_exp activation table (serves Ln, Exp, Copy)
        nc.scalar.activation(tpre[:], epsb[:], AF.Ln, bias=epsb[:])

        V = nc.vector
        P = nc.gpsimd

        # x DMAs: quadrant copy (H-matmul stationaries) + duplicated compact
        # copy (s0-matmul stationary, on the Pool lane)
        for r in range(4):
            eng = nc.sync if r < 2 else nc.scalar
            eng.dma_start(ilhs_t[32 * r : 32 * r + 8, :], ilhs_d[8 * r : 8 * r + 8, :])
        nc.gpsimd.dma_start(ilhs2_t[:], ilhs2_d[:])

        Hv = H_t[:].rearrange("p (c d b) -> p c d b", c=CIN, d=D)
        prodv = prod[:].rearrange("p (c d b) -> p c d b", c=CIN, d=D)
        s_v = s_t[:].rearrange("p (d b) -> p d b", d=D)
        L_v = L_t[:].rearrange("p (c b) -> p c b", c=CIN)
        Lh_v = Lh_t[:].rearrange("p (c b) -> p c b", c=CIN)
        E_v = E_t[:].rearrange("p (c b) -> p c b", c=CIN)
        C_v = C_t[:].rearrange("p (c b) -> p c b", c=CIN)
        Dt8v = Dt8[:].rearrange("p (c e) -> p c e", c=CIN)
        Rh2v = Rh2[:].rearrange("p (c e) -> p c e", c=CIN)
        outB_v = outB[:].rearrange("p (d b) -> p d b", d=D)
        outF_v = outF[:].rearrange("p (b d) -> p b d", b=B16)
        sq_v = sq[:].rearrange("p (d b) -> p d b", d=D)

        def squash(s_src, final: bool):
            # s_src: [p, d, b] f32
            nc.vector.tensor_tensor(sq[:], s_t[:], s_t[:], op=ALU.mult)
            for w in (8, 4, 2):
                V.tensor_tensor(
                    sq_v[:, 0:w, :], sq_v[:, 0:w, :], sq_v[:, w : 2 * w, :],
                    op=ALU.add,
                )
            V.tensor_tensor(
                n2[:].unsqueeze(1), sq_v[:, 0:1, :], sq_v[:, 1:2, :], op=ALU.add
            )
            # sqrt via ln/exp: stays in the natural_log_exp act table set, so
            # softmax's Exp never triggers an activation-table reload
            nc.scalar.activation(t1[:], n2[:], AF.Ln, bias=epsb[:])
            nc.scalar.activation(rs[:], t1[:], AF.Exp, scale=0.5)
            # t1 = (n2 + 1) * sqrt(n2 + eps) in one fused op
            nc.vector.scalar_tensor_tensor(
                t1[:], n2[:], 1.0, rs[:], op0=ALU.add, op1=ALU.mult
            )
            nc.vector.reciprocal(r1[:], t1[:])
            nc.vector.tensor_tensor(fac[:], n2[:], r1[:], op=ALU.mult)
            if final:
                # outF is (b, d); iterate (b, d) over the (d, b)-stored s
                s_bd = s_t[:].rearrange("p (d b) -> p b d", d=D)
                facb = fac[:].unsqueeze(2).broadcast_to((128, B16, D))
                V.tensor_tensor(outF_v, s_bd, facb, op=ALU.mult)
            else:
                facb = fac[:].unsqueeze(1).broadcast_to((128, D, B16))
                V.tensor_tensor(outB_v, s_v, facb, op=ALU.mult)

        def b_update(lo, hi, eng, first: bool):
            # prod[:, lo:hi] = H * out (broadcast over c), then d-tree,
            # then Lhat -> L
            ob = outB_v.unsqueeze(1).broadcast_to((128, hi - lo, D, B16))
            eng.tensor_tensor(
                prodv[:, lo:hi], Hv[:, lo:hi], ob, op=ALU.mult
            )
            for w in (8, 4, 2):
                eng.tensor_tensor(
                    prodv[:, lo:hi, 0:w, :],
                    prodv[:, lo:hi, 0:w, :],
                    prodv[:, lo:hi, w : 2 * w, :],
                    op=ALU.add,
                )
            d0 = prodv[:, lo:hi, 0, :]
            d1 = prodv[:, lo:hi, 1, :]
            if first:
                eng.tensor_tensor(L_v[:, lo:hi], d0, d1, op=ALU.add)
            else:
                eng.tensor_tensor(Lh_v[:, lo:hi], d0, d1, op=ALU.add)
                eng.tensor_tensor(
                    L_v[:, lo:hi], L_v[:, lo:hi], Lh_v[:, lo:hi], op=ALU.add
                )

        def softmax():
            # E = exp(L); denom tree over b (innermost); cross-half swap via
            # SBUF DMA; C = E * recip(denom)
            for lo, hi, teng in ((0, CSPLIT, V), (CSPLIT, CIN, P)):
                nc.scalar.activation(E_v[:, lo:hi], L_v[:, lo:hi], AF.Exp)
                teng.tensor_tensor(
                    Dt8v[:, lo:hi], E_v[:, lo:hi, 0:8], E_v[:, lo:hi, 8:16],
                    op=ALU.add,
                )
                for w in (4, 2):
                    teng.tensor_tensor(
                        Dt8v[:, lo:hi, 0:w], Dt8v[:, lo:hi, 0:w],
                        Dt8v[:, lo:hi, w : 2 * w], op=ALU.add,
                    )
                teng.tensor_tensor(
                    Dh[:, lo:hi].unsqueeze(2), Dt8v[:, lo:hi, 0:1],
                    Dt8v[:, lo:hi, 1:2], op=ALU.add,
                )
            # swap halves on the DVE lane shuffle unit (identity mask, offset
            # partition views), then full-width add
            idm = list(range(32))
            V.stream_shuffle(Dtmp[0:64, :], Dh[64:128, :], idm)
            V.stream_shuffle(Dtmp[64:128, :], Dh[0:64, :], idm)
            nc.vector.tensor_tensor(Dh[:], Dh[:], Dtmp[:], op=ALU.add)
            with nc.allow_low_precision(
                reason="softmax coefficients are bf16 throughout"
            ):
                nc.vector.reciprocal(Rh[:], Dh[:])
            # duplicate recip per c into pairs so the C-multiply keeps an
            # innermost stride-1 pair (2x mode)
            P.tensor_copy(Rh2v, Rh[:].unsqueeze(2).broadcast_to((128, CIN, 2)))
            rb = Rh2v.unsqueeze(2).broadcast_to((128, CIN, 8, 2))
            E_p = E_t[:].rearrange("p (c e two) -> p c e two", c=CIN, two=2)
            C_p = C_t[:].rearrange("p (c e two) -> p c e two", c=CIN, two=2)
            V.tensor_tensor(C_p[:, 0:CSPLIT], E_p[:, 0:CSPLIT], rb[:, 0:CSPLIT], op=ALU.mult)
            P.tensor_tensor(C_p[:, CSPLIT:CIN], E_p[:, CSPLIT:CIN], rb[:, CSPLIT:CIN], op=ALU.mult)

        def s_phase():
            # prod = C*H (broadcast over d), c-trees per engine, combine
            cb = C_v.unsqueeze(2).broadcast_to((128, CIN, D, B16))
            V.tensor_tensor(
                prodv[:, 0:CSPLIT], Hv[:, 0:CSPLIT], cb[:, 0:CSPLIT], op=ALU.mult
            )
            P.tensor_tensor(
                prodv[:, CSPLIT:CIN], Hv[:, CSPLIT:CIN], cb[:, CSPLIT:CIN],
                op=ALU.mult,
            )
            _halving_tree(V, prodv, 0, CSPLIT, 256)
            _halving_tree(P, prodv, CSPLIT, CIN - CSPLIT, 256)
            V.tensor_tensor(
                s_v, prodv[:, 0], prodv[:, CSPLIT], op=ALU.add
            )

        # ---- H generation + s0 matmuls + iter-0 b-update, pipelined ----
        with (
            tc.tile_pool(name="w", bufs=2) as wpool,
            tc.tile_pool(name="w2", bufs=2) as w2pool,
            tc.tile_pool(name="psum", bufs=3, space="PSUM") as pp,
            tc.tile_pool(name="psum0", bufs=1, space="PSUM") as pp0,
        ):
            ps0 = pp0.tile([128, 512], F32, tag="s0")
            NCH = 9  # W stream chunks
            CQC = CQ // NCH  # cq per chunk
            CHW = CQC * 512
            drain_rot = 0
            for h in range(NCH):
                wc = wpool.tile([128, CHW], BF16)
                wc2 = w2pool.tile([32, CHW], BF16)
                # quadrant rows: r0,r1 on SP, r2 on Act, r3 on Pool;
                # compact copy alternates SP/Act after the quadrant rows
                for r, eng in ((0, nc.sync), (1, nc.sync), (2, nc.gpsimd), (3, nc.gpsimd)):
                    eng.dma_start(
                        wc[32 * r : 32 * r + 8, :],
                        wrhs_d[8 * r : 8 * r + 8, h * CHW : (h + 1) * CHW],
                    )
                (nc.scalar if h % 2 == 0 else nc.sync).dma_start(
                    wc2[:], wrhs_d[:, h * CHW : (h + 1) * CHW]
                )  # compact W on Act/SP
                for cql in range(CQC):
                    cq = h * CQC + cql
                    # s0 accumulation matmul (K=32): sum_{c in cq, i} x*W
                    nc.tensor.matmul(
                        ps0[:, :],
                        ilhs2_t[:, cq * 128 : (cq + 1) * 128],
                        wc2[:, cql * 512 : (cql + 1) * 512],
                        start=(cq == 0),
                        stop=(cq == CQ - 1),
                        tile_position=(0, 0),
                        skip_group_check=True,
                    )
                    lhs = ilhs_t[:, cq * BS : (cq + 1) * BS]
                    # paired PSUM: one [128,1024] tile = 2 banks = 2 adjacent c
                    for rp in range(2):
                        pts = pp.tile([128, 1024], F32, tag="ptsm")
                        for rr in range(2):
                            r = 2 * rp + rr
                            for b0 in range(2):
                                rhs = wc[
                                    32 * r : 32 * r + 8,
                                    cql * 512 + b0 * 256 : cql * 512 + b0 * 256 + 256,
                                ]
                                nc.tensor.matmul(
                                    pts[b0 * 64 : (b0 + 1) * 64, 512 * rr : 512 * rr + 256],
                                    lhs[32 * r : 32 * r + 8, :],
                                    rhs,
                                    start=True,
                                    stop=True,
                                    tile_position=(32 * r, b0 * 64),
                                )
                        c = 4 * cq + 2 * rp
                        dsts = Hv[:, c : c + 2].rearrange("p c d b -> p c (d b)")
                        srcs = pts[:].rearrange("p (two half) -> p two half", two=2)[
                            :, :, 0:256
                        ]
                        # drain rotation (GPSIMD cannot access PSUM on HW):
                        # DVE 5/9, Act 4/9
                        k = drain_rot % 9
                        drain_rot += 1
                        if k < 5:
                            V.tensor_copy(dsts, srcs)
                        else:
                            nc.scalar.copy(dsts, srcs)
            # s0 drain: each partition half reads its own b0 column block
            V.tensor_scalar(
                s_t[0:64, :], ps0[0:64, 0:256], 1.0 / NCAP, None, op0=ALU.mult
            )
            V.tensor_scalar(
                s_t[64:128, :], ps0[64:128, 256:512], 1.0 / NCAP, None, op0=ALU.mult
            )
            squash(s_v, final=False)
            # iter-0 b-update: single full-width op per engine (all of H is
            # drained by the time out0 exists, so chunking buys no overlap)
            b_update(0, CSPLIT, V, first=True)
            b_update(CSPLIT, CIN, P, first=True)

        # ---- routing iterations 1, 2 ----
        softmax()
        s_phase()
        squash(s_v, final=False)
        b_update(0, CSPLIT, V, first=False)
        b_update(CSPLIT, CIN, P, first=False)
        softmax()
        s_phase()
        squash(s_v, final=True)

        for b0 in range(2):
            oap = out_d[:, b0 * B16 : (b0 + 1) * B16, :].rearrange(
                "a b d -> a (b d)"
            )
            nc.sync.dma_start(oap, outF[b0 * 64 : (b0 + 1) * 64, :])

    # The TRN2 matmul ISA encoding only fits one sync wait; Tile can emit
    # several. Run the bacc fix-up passes: excess matmul waits move to the
    # paired ldweights, and any instruction still holding >1 wait gets them
    # split into preceding EventSemaphore instructions.
    import bass_rust as _bass_rust

    _bass_rust.move_matmul_waits_to_ldweights(nc.m)
    _bass_rust.generate_event_semaphores(nc)
    return nc


def _bf16(x: np.ndarray) -> np.ndarray:
    import ml_dtypes

    return x.astype(ml_dtypes.bfloat16)


def _pack_w(W: np.ndarray) -> np.ndarray:
    # wrhs[8r+i, cq*512 + b0*256 + j*16 + bm] = W[b0*16+bm, 4cq+r, j, i]
    wrhs = np.empty((32, CQ * 512), np.float32)
    for r in range(4):
        blk = W[:, r::4, :, :]  # [b, cq, j, i]
        blk = blk.reshape(2, B16, CQ, D, I8)  # [b0, bm, cq, j, i]
        wrhs[8 * r : 8 * r + 8, :] = np.ascontiguousarray(
            blk.transpose(4, 2, 0, 3, 1)  # [i, cq, b0, j, bm]
        ).reshape(8, CQ * 512)
    return _bf16(wrhs)


def _pack_x(xs: np.ndarray) -> np.ndarray:
    # ilhs[8r+i, cq*64 + a] = xs[a, 4cq+r, i]
    ilhs = np.empty((32, CQ * BS), np.float32)
    for r in range(4):
        blk = xs[:, r::4, :]  # [a, cq, i]
        ilhs[8 * r : 8 * r + 8, :] = np.ascontiguousarray(
            blk.transpose(2, 1, 0)
        ).reshape(8, CQ * BS)
    return _bf16(ilhs)


def _pack_x2(xs: np.ndarray) -> np.ndarray:
    # ilhs2[8r+i, cq*128 + b0*64 + a] = xs[a, 4cq+r, i]  (duplicated over b0)
    ilhs = np.empty((32, CQ, BS), np.float32)
    for r in range(4):
        blk = xs[:, r::4, :]  # [a, cq, i]
        ilhs[8 * r : 8 * r + 8] = blk.transpose(2, 1, 0)
    ilhs2 = np.concatenate([ilhs[:, :, None, :], ilhs[:, :, None, :]], axis=2)
    return _bf16(ilhs2.reshape(32, CQ * 128))


_CACHED = {}


def _get_program():
    if "nc" not in _CACHED:
        _CACHED["nc"] = _build_program()
    return _CACHED["nc"]


def _per_core_inputs(inputs: np.ndarray, W: np.ndarray) -> list:
    wrhs = _pack_w(W)
    in_maps = []
    for k in range(NCORES):
        xs = inputs[k * BS : (k + 1) * BS]
        in_maps.append(
            {"ilhs": _pack_x(xs), "ilhs2": _pack_x2(xs), "wrhs": wrhs}
        )
    return in_maps


def kernel(inputs: np.ndarray, W: np.ndarray) -> np.ndarray:
    inputs = np.asarray(inputs, np.float32)
    W = np.asarray(W, np.float32)
    nc = _get_program()
    in_maps = _per_core_inputs(inputs, W)
    res = run_bass_kernel_spmd(nc, in_maps, core_ids=list(range(NCORES)))
    out = np.concatenate([res.results[k]["out"] for k in range(NCORES)], axis=0)
    return out
